# revision 34
# baseline (speedup 1.0000x reference)
"""Multi-head attention (RoPE) Trainium2 kernel.

Problem: B=2, T=2048, D_MODEL=1024, 16 heads x d_k=64, fp32 in/out.

Sharding: tensor-parallel over heads. Core c owns heads 2c, 2c+1:
  - wq/wk/wv rows [128c, 128c+128)  (column-split of the projections)
  - wo columns [128c, 128c+128)     (row-split of the output projection)
Each core computes, per head, an UNNORMALIZED full-shape partial of the
output projection plus the softmax denominators; the host applies the
denominators and sums the 16 partials (the "all-reduce" of row-parallel wo).

On-chip dataflow per core (fp16 matmul operands, fp32 PSUM):
  xT [D=1024, tok=4096] (token-major b*2048+s) @ wT slices -> QT/KT/VT [128, 4096]
  RoPE on QT/KT in [d', tok] layout per 1024-token chunk (tables precomputed
  host-side, partition swap via SBUF-SBUF DMA).
  V transposed per 128-token tile on the PE to [tok, 64]-per-head tiles with
  a ones column appended (the 65th stationary column makes the AV matmul
  accumulate the softmax denominator into PSUM row 64 for free).
  Scores ST[k, q] = K @ Q^T per head; the d_k=64 contraction means the two
  heads run row-tiled ((0,0)/(64,0)) concurrently on the PE.
  exp on ScalarE (scale=1/8 folded in; no max-subtraction: scores ~ N(0,1)).
  Output projection per head, row-tiled (contraction d=64): concurrent
  matmul pairs producing OUT_A^T / OUT_B^T, evicted fp32 to HBM unnormalized.

All PSUM compute tiles are one bank ([128,512] f32) rotating through 4 slots
so exp(kt) overlaps the scores of kt+1; the two AV accumulators [65,1024]
hold the other 4 banks. Phase P is interleaved with attention: batch 0's
attention is emitted after the first half of the projections.
"""

import sys

sys.path.insert(0, "/opt/trn_rl_repo")

import numpy as np

import concourse.bacc as bacc
import concourse.bass as bass
import concourse.tile as tile
from concourse import mybir
from concourse.masks import make_identity

F16 = mybir.dt.float16
F32 = mybir.dt.float32

B = 2
T = 2048
D = 1024
NTOK = B * T  # 4096
DK = 64
N_CORES = 8
QCH = 1024  # query chunk (per (b, qh))
KT_N = T // 128  # 16 key tiles per batch


def _build_body(tc, xT, wqT, wkT, wvT, woT, ropeA, ropeB, outTA, outTB, dens):
    nc = tc.nc
    Exp = mybir.ActivationFunctionType.Exp

    const = tc.alloc_tile_pool(name="const", bufs=1)
    psum = tc.alloc_tile_pool(name="psum", bufs=1, space="PSUM")

    # ---------------- persistent tiles ----------------
    w_sb = {}
    for nm, w in (("wq", wqT), ("wk", wkT), ("wv", wvT)):
        wt = const.tile([128, 8, 128], F16, name=f"{nm}sb")
        nc.sync.dma_start(out=wt, in_=w.rearrange("(a p) m -> p a m", p=128))
        w_sb[nm] = wt
    wo_sb = const.tile([128, 1024], F16)
    nc.sync.dma_start(out=wo_sb, in_=woT)
    rA = const.tile([128, 4096], F16)
    nc.sync.dma_start(out=rA, in_=ropeA)
    rB = const.tile([128, 4096], F16)
    nc.sync.dma_start(out=rB, in_=ropeB)
    ident = const.tile([128, 128], F16)
    make_identity(nc, ident)

    q_rot = const.tile([128, 4096], F16)
    k_rot = const.tile([128, 4096], F16)
    # per 128-token tile, per head: [V(0:64) | ones(64) | pad] fp16
    v_sb = [
        [const.tile([128, 72], F16, name=f"vsb{i}h{h}") for h in range(2)]
        for i in range(NTOK // 128)
    ]
    for vpair in v_sb:
        for vt in vpair:
            nc.vector.memset(vt, 1.0)

    at = tc.alloc_tile_pool(name="attn", bufs=1)
    pp = tc.alloc_tile_pool(name="phasep", bufs=1)

    # ---------------- phase P (emitted interleaved with attention) ----------
    xs = [pp.tile([128, 4096], F16, name=f"xs{k}") for k in range(8)]
    for t4 in range(4):
        for k in range(8):
            cs = slice(t4 * 1024, (t4 + 1) * 1024)
            nc.sync.dma_start(out=xs[k][:, cs], in_=xT[k * 128 : (k + 1) * 128, cs])
    vt_raw = pp.tile([128, 4096], F16)

    def proj_chunk(wt, dst, t4):
        for h2 in range(2):
            cs = slice(t4 * 1024 + h2 * 512, t4 * 1024 + (h2 + 1) * 512)
            ps = psum.tile([128, 512], F32, tag="s", bufs=4, name="ps_pr")
            for k in range(8):
                nc.tensor.matmul(
                    ps,
                    lhsT=wt[:, k, :],
                    rhs=xs[k][:, cs],
                    start=(k == 0),
                    stop=(k == 7),
                )
            nc.vector.tensor_copy(dst[:, cs], ps)

    def rope_chunk(raw, t4):
        # out = raw*A + swap(raw)*B, swap = +-32 partitions within a head
        cs = slice(t4 * 1024, (t4 + 1) * 1024)
        sw = pp.tile([128, 1024], F16, tag="sw", bufs=2, name="ropesw")
        for dst_p, src_p in ((0, 32), (32, 0), (64, 96), (96, 64)):
            nc.sync.dma_start(
                out=sw[dst_p : dst_p + 32, :], in_=raw[src_p : src_p + 32, cs]
            )
        t1 = pp.tile([128, 1024], F16, tag="t1", bufs=2, name="ropet1")
        nc.vector.tensor_mul(t1, raw[:, cs], rA[:, cs])
        nc.vector.tensor_mul(sw, sw, rB[:, cs])
        nc.vector.tensor_add(raw[:, cs], t1, sw)

    def v_chunk_transpose(t4):
        # V transpose on the PE: vt_raw [d', tok] -> v_sb [tok128, d64]
        for i in range(8 * t4, 8 * (t4 + 1)):
            ts = slice(i * 128, (i + 1) * 128)
            pst = psum.tile([128, 512], F32, tag="s", bufs=4, name="ps_tr")
            tr = pst[:, 0:64].bitcast(F16)  # [128, 128] f16 view
            nc.tensor.transpose(tr, vt_raw[:, ts], ident)
            nc.vector.tensor_copy(v_sb[i][0][:, 0:64], tr[:, 0:64])
            nc.vector.tensor_copy(v_sb[i][1][:, 0:64], tr[:, 64:128])

    def phase_p(t4):
        proj_chunk(w_sb["wq"], q_rot, t4)
        rope_chunk(q_rot, t4)
        proj_chunk(w_sb["wk"], k_rot, t4)
        rope_chunk(k_rot, t4)
        proj_chunk(w_sb["wv"], vt_raw, t4)
        v_chunk_transpose(t4)

    # ---------------- attention ----------------
    pending_oproj = [None]

    def flush_oproj():
        if pending_oproj[0] is not None:
            pending_oproj[0]()
            pending_oproj[0] = None

    def chunk(b, qh):
        qoff = b * T + qh * QCH
        crow = 2 * b + qh  # chunk index 0..3

        exp_tiles = {}

        def s_exp(kt, h2):
            # the two heads' score matmuls are row-tiled (PE rows 0:64 /
            # 64:128) and run concurrently when emitted adjacently.
            koff = b * T + kt * 128
            pss = []
            for hi in range(2):
                ps_s = psum.tile([128, 512], F32, tag="s", bufs=4, name="ps_s")
                pss.append(ps_s)
            for hi in range(2):
                hs = slice(64 * hi, 64 * hi + 64)
                nc.tensor.matmul(
                    pss[hi],
                    lhsT=k_rot[hs, koff : koff + 128],
                    rhs=q_rot[hs, qoff + h2 * 512 : qoff + (h2 + 1) * 512],
                    start=True,
                    stop=True,
                )
            for hi in range(2):
                e = at.tile([128, 512], F16, tag="exp", bufs=10, name="exps")
                nc.scalar.activation(e, pss[hi], Exp, scale=0.125)
                exp_tiles[(hi, kt, h2)] = e

        ps_o = [
            psum.tile([65, 1024], F32, tag="o", bufs=2, name=f"ps_o{hi}")
            for hi in range(2)
        ]

        def av(kt, h2):
            vt = v_sb[b * KT_N + kt]
            for hi in range(2):
                e = exp_tiles.pop((hi, kt, h2))
                nc.tensor.matmul(
                    ps_o[hi][:, h2 * 512 : (h2 + 1) * 512],
                    lhsT=vt[hi][:, 0:65],
                    rhs=e,
                    start=(kt == 0),
                    stop=(kt == KT_N - 1),
                    skip_group_check=True,
                )

        # pipelined emission with one-step lag so the PE never waits on exp
        steps = [(kt, h2) for kt in range(KT_N) for h2 in range(2)]
        s_exp(*steps[0])
        s_exp(*steps[1])
        flush_oproj()
        av(*steps[0])
        for i in range(2, len(steps)):
            s_exp(*steps[i])
            av(*steps[i - 1])
        av(*steps[-1])

        # evict unnormalized O^T per head + denominators
        ocat = at.tile([128, 1024], F16, tag="ocat", bufs=2, name="ocat")
        nc.vector.tensor_copy(ocat[0:64, :], ps_o[0][0:64, :])
        oBt = at.tile([64, 1024], F16, tag="oBt", bufs=2, name="oBt")
        nc.vector.tensor_copy(oBt, ps_o[1][0:64, :])
        nc.sync.dma_start(out=ocat[64:128, :], in_=oBt)
        for hi in range(2):
            dent = at.tile([1, 1024], F32, tag="dent", bufs=2, name="dent")
            nc.vector.tensor_copy(dent, ps_o[hi][64:65, :])
            nc.sync.dma_start(
                out=dens[2 * crow + hi : 2 * crow + hi + 1, :], in_=dent
            )

        def oproj():
            for nt in range(8):
                nts = slice(nt * 128, (nt + 1) * 128)
                ot = [
                    at.tile([128, 1024], F32, tag=f"ot{hi}", bufs=2, name=f"ot{hi}")
                    for hi in range(2)
                ]
                for h2 in range(2):
                    h2s = slice(h2 * 512, (h2 + 1) * 512)
                    ps_u = []
                    for hi in range(2):
                        u = psum.tile([128, 512], F32, tag="s", bufs=4, name="ps_u")
                        ps_u.append(u)
                    for hi in range(2):
                        hs = slice(64 * hi, 64 * hi + 64)
                        nc.tensor.matmul(
                            ps_u[hi],
                            lhsT=wo_sb[hs, nts],
                            rhs=ocat[hs, h2s],
                            start=True,
                            stop=True,
                        )
                    for hi in range(2):
                        nc.vector.tensor_copy(ot[hi][:, h2s], ps_u[hi])
                for hi, outT in ((0, outTA), (1, outTB)):
                    nc.sync.dma_start(
                        out=outT[nts, qoff : qoff + QCH], in_=ot[hi]
                    )

        pending_oproj[0] = oproj

    # ---------------- schedule ----------------
    phase_p(0)
    phase_p(1)
    chunk(0, 0)
    phase_p(2)
    chunk(0, 1)
    phase_p(3)
    chunk(1, 0)
    chunk(1, 1)
    flush_oproj()

    pp.release()
    at.release()
    const.release()
    psum.release()


_NC_CACHE = {}


def _build_program():
    if 0 in _NC_CACHE:
        return _NC_CACHE[0]
    nc = bacc.Bacc("TRN2", num_devices=N_CORES, debug=False)
    xT = nc.dram_tensor("xT", [D, NTOK], F16, kind="ExternalInput").ap()
    wqT = nc.dram_tensor("wqT", [D, 128], F16, kind="ExternalInput").ap()
    wkT = nc.dram_tensor("wkT", [D, 128], F16, kind="ExternalInput").ap()
    wvT = nc.dram_tensor("wvT", [D, 128], F16, kind="ExternalInput").ap()
    woT = nc.dram_tensor("woT", [128, D], F16, kind="ExternalInput").ap()
    ropeA = nc.dram_tensor("ropeA", [128, NTOK], F16, kind="ExternalInput").ap()
    ropeB = nc.dram_tensor("ropeB", [128, NTOK], F16, kind="ExternalInput").ap()
    outTA = nc.dram_tensor("outTA", [D, NTOK], F32, kind="ExternalOutput").ap()
    outTB = nc.dram_tensor("outTB", [D, NTOK], F32, kind="ExternalOutput").ap()
    dens = nc.dram_tensor("dens", [8, QCH], F32, kind="ExternalOutput").ap()
    with tile.TileContext(nc) as tc:
        _build_body(tc, xT, wqT, wkT, wvT, woT, ropeA, ropeB, outTA, outTB, dens)
    nc.compile()
    _NC_CACHE[0] = nc
    return nc


def _rope_tables():
    half = DK // 2  # 32
    inv_freq = 1.0 / (
        10000.0 ** (np.arange(0, DK, 2, dtype=np.float32) / np.float32(DK))
    )
    t = np.arange(T, dtype=np.float32)
    freqs = np.outer(t, inv_freq)  # [T, 32]
    cos = np.cos(freqs)
    sin = np.sin(freqs)
    A = np.empty((128, NTOK), np.float32)
    Bt = np.empty((128, NTOK), np.float32)
    for p in range(128):
        i = p % DK
        if i < half:
            a, bb = cos[:, i], -sin[:, i]
        else:
            a, bb = cos[:, i - half], sin[:, i - half]
        for bi in range(B):
            A[p, bi * T : (bi + 1) * T] = a
            Bt[p, bi * T : (bi + 1) * T] = bb
    return A.astype(np.float16), Bt.astype(np.float16)


def _prep_inputs(x, wq, wk, wv, wo):
    xT = np.ascontiguousarray(x.reshape(NTOK, D).T).astype(np.float16)
    ropeA, ropeB = _rope_tables()
    in_maps = []
    for c in range(N_CORES):
        rows = slice(128 * c, 128 * (c + 1))
        in_maps.append(
            {
                "xT": xT,
                "wqT": np.ascontiguousarray(wq[rows, :].T).astype(np.float16),
                "wkT": np.ascontiguousarray(wk[rows, :].T).astype(np.float16),
                "wvT": np.ascontiguousarray(wv[rows, :].T).astype(np.float16),
                "woT": np.ascontiguousarray(wo[:, rows].T).astype(np.float16),
                "ropeA": ropeA,
                "ropeB": ropeB,
            }
        )
    return in_maps


def run(x, wq, wk, wv, wo, trace=False):
    """Returns (output (B,T,D) fp32, BassKernelResults)."""
    from concourse import bass_utils

    nc = _build_program()
    in_maps = _prep_inputs(
        np.asarray(x, np.float32),
        np.asarray(wq, np.float32),
        np.asarray(wk, np.float32),
        np.asarray(wv, np.float32),
        np.asarray(wo, np.float32),
    )
    res = bass_utils.run_bass_kernel_spmd(
        nc, in_maps, core_ids=list(range(N_CORES)), trace=trace
    )
    acc = np.zeros((D, NTOK), np.float32)
    for c in range(N_CORES):
        r = res.results[c]
        dens_ = np.asarray(r["dens"], np.float32)  # [8, 1024]
        rec = np.empty((2, NTOK), np.float32)
        for b in range(B):
            for qh in range(2):
                crow = 2 * b + qh
                qoff = b * T + qh * QCH
                for hi in range(2):
                    rec[hi, qoff : qoff + QCH] = 1.0 / dens_[2 * crow + hi]
        acc += np.asarray(r["outTA"], np.float32) * rec[0][None, :]
        acc += np.asarray(r["outTB"], np.float32) * rec[1][None, :]
    out = acc.T.reshape(B, T, D)
    return out, res


def kernel(x, wq, wk, wv, wo):
    out, _ = run(x, wq, wk, wv, wo)
    return out


# revision 39
# speedup vs baseline: 1.2671x; 1.2671x over previous
"""Multi-head attention (RoPE) Trainium2 kernel.

Problem: B=2, T=2048, D_MODEL=1024, 16 heads x d_k=64, fp32 in/out.

Sharding: tensor-parallel over heads. Core c owns heads 2c, 2c+1:
  - wq/wk/wv rows [128c, 128c+128)  (column-split of the projections)
  - wo columns [128c, 128c+128)     (row-split of the output projection)
Each core computes, per head, an UNNORMALIZED full-shape partial of the
output projection plus the softmax denominators; the host applies the
denominators and sums the 16 partials (the "all-reduce" of row-parallel wo).

On-chip dataflow per core (fp16 matmul operands, fp32 PSUM):
  xT [D=1024, tok=4096] (token-major b*2048+s) @ wT slices -> QT/KT/VT [128, 4096]
  RoPE on QT/KT in [d', tok] layout per 1024-token chunk (tables precomputed
  host-side, partition swap via SBUF-SBUF DMA).
  V transposed per 128-token tile on the PE to [tok, 64]-per-head tiles with
  a ones column appended (the 65th stationary column makes the AV matmul
  accumulate the softmax denominator into PSUM row 64 for free).
  Scores ST[k, q] = K @ Q^T per head; the d_k=64 contraction means the two
  heads run row-tiled ((0,0)/(64,0)) concurrently on the PE.
  exp on ScalarE (scale=1/8 folded in; no max-subtraction: scores ~ N(0,1)).
  Output projection per head, row-tiled (contraction d=64): concurrent
  matmul pairs producing OUT_A^T / OUT_B^T, evicted fp32 to HBM unnormalized.

All PSUM compute tiles are one bank ([128,512] f32) rotating through 4 slots
so exp(kt) overlaps the scores of kt+1; the two AV accumulators [65,1024]
hold the other 4 banks. Phase P is interleaved with attention: batch 0's
attention is emitted after the first half of the projections.
"""

import sys

sys.path.insert(0, "/opt/trn_rl_repo")

import numpy as np

import concourse.bacc as bacc
import concourse.bass as bass
import concourse.tile as tile
from concourse import mybir
from concourse.masks import make_identity

F16 = mybir.dt.float16
F32 = mybir.dt.float32

B = 2
T = 2048
D = 1024
NTOK = B * T  # 4096
DK = 64
N_CORES = 8
QCH = 1024  # query chunk (per (b, qh))
KT_N = T // 128  # 16 key tiles per batch


def _build_body(tc, xT, wqT, wkT, wvT, woT, ropeA, ropeB, outTA, outTB, dens):
    nc = tc.nc
    Exp = mybir.ActivationFunctionType.Exp

    const = tc.alloc_tile_pool(name="const", bufs=1)
    psum = tc.alloc_tile_pool(name="psum", bufs=1, space="PSUM")

    # ---------------- persistent tiles ----------------
    w_sb = {}
    for nm, w in (("wq", wqT), ("wk", wkT), ("wv", wvT)):
        wt = const.tile([128, 8, 128], F16, name=f"{nm}sb")
        nc.sync.dma_start(out=wt, in_=w.rearrange("(a p) m -> p a m", p=128))
        w_sb[nm] = wt
    wo_sb = const.tile([128, 1024], F16)
    nc.sync.dma_start(out=wo_sb, in_=woT)
    rA = const.tile([128, 4096], F16)
    nc.sync.dma_start(out=rA, in_=ropeA)
    rB = const.tile([128, 4096], F16)
    nc.sync.dma_start(out=rB, in_=ropeB)
    ident = const.tile([128, 128], F16)
    make_identity(nc, ident)

    q_rot = const.tile([128, 4096], F16)
    k_rot = const.tile([128, 4096], F16)
    # per 128-token tile, per head: [V(0:64) | ones(64) | pad] fp16
    v_sb = [
        [const.tile([128, 72], F16, name=f"vsb{i}h{h}") for h in range(2)]
        for i in range(NTOK // 128)
    ]
    for vpair in v_sb:
        for vt in vpair:
            nc.vector.memset(vt, 1.0)

    at = tc.alloc_tile_pool(name="attn", bufs=1)
    pp = tc.alloc_tile_pool(name="phasep", bufs=1)

    # ---------------- phase P (emitted interleaved with attention) ----------
    xs = [pp.tile([128, 4096], F16, name=f"xs{k}") for k in range(8)]
    for t4 in range(4):
        for k in range(8):
            cs = slice(t4 * 1024, (t4 + 1) * 1024)
            nc.sync.dma_start(out=xs[k][:, cs], in_=xT[k * 128 : (k + 1) * 128, cs])
    vt_raw = pp.tile([128, 4096], F16)

    def proj_chunk(wt, dst, t4):
        cs = slice(t4 * 1024, (t4 + 1) * 1024)
        ps = psum.tile([128, 1024], F32, tag="mm", bufs=2, name="ps_pr")
        for k in range(8):
            for h2 in range(2):
                nc.tensor.matmul(
                    ps[:, h2 * 512 : (h2 + 1) * 512],
                    lhsT=wt[:, k, :],
                    rhs=xs[k][:, t4 * 1024 + h2 * 512 : t4 * 1024 + (h2 + 1) * 512],
                    start=(k == 0),
                    stop=(k == 7),
                )
        nc.vector.tensor_copy(dst[:, cs], ps)

    def rope_chunk(raw, t4):
        # out = raw*A + swap(raw)*B, swap = +-32 partitions within a head
        cs = slice(t4 * 1024, (t4 + 1) * 1024)
        sw = pp.tile([128, 1024], F16, tag="sw", bufs=2, name="ropesw")
        for dst_p, src_p in ((0, 32), (32, 0), (64, 96), (96, 64)):
            nc.sync.dma_start(
                out=sw[dst_p : dst_p + 32, :], in_=raw[src_p : src_p + 32, cs]
            )
        t1 = pp.tile([128, 1024], F16, tag="t1", bufs=2, name="ropet1")
        nc.vector.tensor_mul(t1, raw[:, cs], rA[:, cs])
        nc.vector.tensor_mul(sw, sw, rB[:, cs])
        nc.vector.tensor_add(raw[:, cs], t1, sw)

    def v_chunk_transpose(t4):
        # V transpose on the PE: vt_raw [d', tok] -> v_sb [tok128, d64]
        for i in range(8 * t4, 8 * (t4 + 1)):
            ts = slice(i * 128, (i + 1) * 128)
            pst = psum.tile([128, 1024], F32, tag="mm", bufs=2, name="ps_tr")
            tr = pst[:, 0:64].bitcast(F16)  # [128, 128] f16 view
            nc.tensor.transpose(tr, vt_raw[:, ts], ident)
            nc.vector.tensor_copy(v_sb[i][0][:, 0:64], tr[:, 0:64])
            nc.vector.tensor_copy(v_sb[i][1][:, 0:64], tr[:, 64:128])

    def phase_p(t4):
        proj_chunk(w_sb["wq"], q_rot, t4)
        rope_chunk(q_rot, t4)
        proj_chunk(w_sb["wk"], k_rot, t4)
        rope_chunk(k_rot, t4)
        proj_chunk(w_sb["wv"], vt_raw, t4)
        v_chunk_transpose(t4)

    # ---------------- attention ----------------
    pending_oproj = [None]

    def flush_oproj():
        if pending_oproj[0] is not None:
            pending_oproj[0]()
            pending_oproj[0] = None

    def chunk(b, qh):
        qoff = b * T + qh * QCH
        crow = 2 * b + qh  # chunk index 0..3

        exp_tiles = {}

        def s_exp(kt):
            # the two heads' score matmuls are row-tiled (PE rows 0:64 /
            # 64:128) and run concurrently when emitted adjacently.
            koff = b * T + kt * 128
            pss = [
                psum.tile([128, 1024], F32, tag="mm", bufs=2, name=f"ps_s{hi}")
                for hi in range(2)
            ]
            for h2 in range(2):
                for hi in range(2):
                    hs = slice(64 * hi, 64 * hi + 64)
                    nc.tensor.matmul(
                        pss[hi][:, h2 * 512 : (h2 + 1) * 512],
                        lhsT=k_rot[hs, koff : koff + 128],
                        rhs=q_rot[hs, qoff + h2 * 512 : qoff + (h2 + 1) * 512],
                        start=True,
                        stop=True,
                    )
            for hi in range(2):
                e = at.tile([128, 1024], F16, tag="exp", bufs=8, name="exps")
                nc.scalar.activation(e, pss[hi], Exp, scale=0.125)
                exp_tiles[(hi, kt)] = e

        ps_o = [
            psum.tile([65, 1024], F32, tag="o", bufs=2, name=f"ps_o{hi}")
            for hi in range(2)
        ]

        def av(kt):
            vt = v_sb[b * KT_N + kt]
            for hi in range(2):
                e = exp_tiles.pop((hi, kt))
                for h2 in range(2):
                    h2s = slice(h2 * 512, (h2 + 1) * 512)
                    nc.tensor.matmul(
                        ps_o[hi][:, h2s],
                        lhsT=vt[hi][:, 0:65],
                        rhs=e[:, h2s],
                        start=(kt == 0),
                        stop=(kt == KT_N - 1),
                        skip_group_check=True,
                    )

        # pipelined emission with one-step lag so the PE never waits on exp
        s_exp(0)
        s_exp(1)
        flush_oproj()
        av(0)
        for kt in range(2, KT_N):
            s_exp(kt)
            av(kt - 1)
        av(KT_N - 1)

        # evict unnormalized O^T per head + denominators
        ocat = at.tile([128, 1024], F16, tag="ocat", bufs=2, name="ocat")
        nc.vector.tensor_copy(ocat[0:64, :], ps_o[0][0:64, :])
        oBt = at.tile([64, 1024], F16, tag="oBt", bufs=2, name="oBt")
        nc.vector.tensor_copy(oBt, ps_o[1][0:64, :])
        nc.sync.dma_start(out=ocat[64:128, :], in_=oBt)
        for hi in range(2):
            dent = at.tile([1, 1024], F32, tag="dent", bufs=2, name="dent")
            nc.vector.tensor_copy(dent, ps_o[hi][64:65, :])
            nc.sync.dma_start(
                out=dens[2 * crow + hi : 2 * crow + hi + 1, :], in_=dent
            )

        def oproj():
            for nt in range(8):
                nts = slice(nt * 128, (nt + 1) * 128)
                ot = [
                    at.tile([128, 1024], F32, tag=f"ot{hi}", bufs=2, name=f"ot{hi}")
                    for hi in range(2)
                ]
                ps_u = [
                    psum.tile([128, 1024], F32, tag="o", bufs=2, name=f"ps_u{hi}")
                    for hi in range(2)
                ]
                for h2 in range(2):
                    h2s = slice(h2 * 512, (h2 + 1) * 512)
                    for hi in range(2):
                        hs = slice(64 * hi, 64 * hi + 64)
                        nc.tensor.matmul(
                            ps_u[hi][:, h2s],
                            lhsT=wo_sb[hs, nts],
                            rhs=ocat[hs, h2s],
                            start=True,
                            stop=True,
                        )
                for hi, outT in ((0, outTA), (1, outTB)):
                    nc.vector.tensor_copy(ot[hi], ps_u[hi])
                    nc.sync.dma_start(
                        out=outT[nts, qoff : qoff + QCH], in_=ot[hi]
                    )

        pending_oproj[0] = oproj

    # ---------------- schedule ----------------
    phase_p(0)
    phase_p(1)
    chunk(0, 0)
    phase_p(2)
    chunk(0, 1)
    phase_p(3)
    chunk(1, 0)
    chunk(1, 1)
    flush_oproj()

    pp.release()
    at.release()
    const.release()
    psum.release()


_NC_CACHE = {}


def _build_program():
    if 0 in _NC_CACHE:
        return _NC_CACHE[0]
    nc = bacc.Bacc("TRN2", num_devices=N_CORES, debug=False)
    xT = nc.dram_tensor("xT", [D, NTOK], F16, kind="ExternalInput").ap()
    wqT = nc.dram_tensor("wqT", [D, 128], F16, kind="ExternalInput").ap()
    wkT = nc.dram_tensor("wkT", [D, 128], F16, kind="ExternalInput").ap()
    wvT = nc.dram_tensor("wvT", [D, 128], F16, kind="ExternalInput").ap()
    woT = nc.dram_tensor("woT", [128, D], F16, kind="ExternalInput").ap()
    ropeA = nc.dram_tensor("ropeA", [128, NTOK], F16, kind="ExternalInput").ap()
    ropeB = nc.dram_tensor("ropeB", [128, NTOK], F16, kind="ExternalInput").ap()
    outTA = nc.dram_tensor("outTA", [D, NTOK], F32, kind="ExternalOutput").ap()
    outTB = nc.dram_tensor("outTB", [D, NTOK], F32, kind="ExternalOutput").ap()
    dens = nc.dram_tensor("dens", [8, QCH], F32, kind="ExternalOutput").ap()
    with tile.TileContext(nc) as tc:
        _build_body(tc, xT, wqT, wkT, wvT, woT, ropeA, ropeB, outTA, outTB, dens)
    nc.compile()
    _NC_CACHE[0] = nc
    return nc


def _rope_tables():
    half = DK // 2  # 32
    inv_freq = 1.0 / (
        10000.0 ** (np.arange(0, DK, 2, dtype=np.float32) / np.float32(DK))
    )
    t = np.arange(T, dtype=np.float32)
    freqs = np.outer(t, inv_freq)  # [T, 32]
    cos = np.cos(freqs)
    sin = np.sin(freqs)
    A = np.empty((128, NTOK), np.float32)
    Bt = np.empty((128, NTOK), np.float32)
    for p in range(128):
        i = p % DK
        if i < half:
            a, bb = cos[:, i], -sin[:, i]
        else:
            a, bb = cos[:, i - half], sin[:, i - half]
        for bi in range(B):
            A[p, bi * T : (bi + 1) * T] = a
            Bt[p, bi * T : (bi + 1) * T] = bb
    return A.astype(np.float16), Bt.astype(np.float16)


def _prep_inputs(x, wq, wk, wv, wo):
    xT = np.ascontiguousarray(x.reshape(NTOK, D).T).astype(np.float16)
    ropeA, ropeB = _rope_tables()
    in_maps = []
    for c in range(N_CORES):
        rows = slice(128 * c, 128 * (c + 1))
        in_maps.append(
            {
                "xT": xT,
                "wqT": np.ascontiguousarray(wq[rows, :].T).astype(np.float16),
                "wkT": np.ascontiguousarray(wk[rows, :].T).astype(np.float16),
                "wvT": np.ascontiguousarray(wv[rows, :].T).astype(np.float16),
                "woT": np.ascontiguousarray(wo[:, rows].T).astype(np.float16),
                "ropeA": ropeA,
                "ropeB": ropeB,
            }
        )
    return in_maps


def run(x, wq, wk, wv, wo, trace=False):
    """Returns (output (B,T,D) fp32, BassKernelResults)."""
    from concourse import bass_utils

    nc = _build_program()
    in_maps = _prep_inputs(
        np.asarray(x, np.float32),
        np.asarray(wq, np.float32),
        np.asarray(wk, np.float32),
        np.asarray(wv, np.float32),
        np.asarray(wo, np.float32),
    )
    res = bass_utils.run_bass_kernel_spmd(
        nc, in_maps, core_ids=list(range(N_CORES)), trace=trace
    )
    acc = np.zeros((D, NTOK), np.float32)
    for c in range(N_CORES):
        r = res.results[c]
        dens_ = np.asarray(r["dens"], np.float32)  # [8, 1024]
        rec = np.empty((2, NTOK), np.float32)
        for b in range(B):
            for qh in range(2):
                crow = 2 * b + qh
                qoff = b * T + qh * QCH
                for hi in range(2):
                    rec[hi, qoff : qoff + QCH] = 1.0 / dens_[2 * crow + hi]
        acc += np.asarray(r["outTA"], np.float32) * rec[0][None, :]
        acc += np.asarray(r["outTB"], np.float32) * rec[1][None, :]
    out = acc.T.reshape(B, T, D)
    return out, res


def kernel(x, wq, wk, wv, wo):
    out, _ = run(x, wq, wk, wv, wo)
    return out


# revision 40
# speedup vs baseline: 1.2912x; 1.0190x over previous
"""Multi-head attention (RoPE) Trainium2 kernel.

Problem: B=2, T=2048, D_MODEL=1024, 16 heads x d_k=64, fp32 in/out.

Sharding: tensor-parallel over heads. Core c owns heads 2c, 2c+1:
  - wq/wk/wv rows [128c, 128c+128)  (column-split of the projections)
  - wo columns [128c, 128c+128)     (row-split of the output projection)
Each core computes, per head, an UNNORMALIZED full-shape partial of the
output projection plus the softmax denominators; the host applies the
denominators and sums the 16 partials (the "all-reduce" of row-parallel wo).

On-chip dataflow per core (fp16 matmul operands, fp32 PSUM):
  xT [D=1024, tok=4096] (token-major b*2048+s) @ wT slices -> QT/KT/VT [128, 4096]
  RoPE on QT/KT in [d', tok] layout per 1024-token chunk (tables precomputed
  host-side, partition swap via SBUF-SBUF DMA).
  V transposed per 128-token tile on the PE to [tok, 64]-per-head tiles with
  a ones column appended (the 65th stationary column makes the AV matmul
  accumulate the softmax denominator into PSUM row 64 for free).
  Scores ST[k, q] = K @ Q^T per head; the d_k=64 contraction means the two
  heads run row-tiled ((0,0)/(64,0)) concurrently on the PE.
  exp on ScalarE (scale=1/8 folded in; no max-subtraction: scores ~ N(0,1)).
  Output projection per head, row-tiled (contraction d=64): concurrent
  matmul pairs producing OUT_A^T / OUT_B^T, evicted fp32 to HBM unnormalized.

All PSUM compute tiles are one bank ([128,512] f32) rotating through 4 slots
so exp(kt) overlaps the scores of kt+1; the two AV accumulators [65,1024]
hold the other 4 banks. Phase P is interleaved with attention: batch 0's
attention is emitted after the first half of the projections.
"""

import sys

sys.path.insert(0, "/opt/trn_rl_repo")

import numpy as np

import concourse.bacc as bacc
import concourse.bass as bass
import concourse.tile as tile
from concourse import mybir
from concourse.masks import make_identity

F16 = mybir.dt.float16
F32 = mybir.dt.float32

B = 2
T = 2048
D = 1024
NTOK = B * T  # 4096
DK = 64
N_CORES = 8
QCH = 1024  # query chunk (per (b, qh))
KT_N = T // 128  # 16 key tiles per batch


def _build_body(tc, xT, wqT, wkT, wvT, woT, ropeA, ropeB, outTA, outTB, dens):
    nc = tc.nc
    Exp = mybir.ActivationFunctionType.Exp

    const = tc.alloc_tile_pool(name="const", bufs=1)
    psum = tc.alloc_tile_pool(name="psum", bufs=1, space="PSUM")

    # ---------------- persistent tiles ----------------
    w_sb = {}
    for nm, w in (("wq", wqT), ("wk", wkT), ("wv", wvT)):
        wt = const.tile([128, 8, 128], F16, name=f"{nm}sb")
        nc.sync.dma_start(out=wt, in_=w.rearrange("(a p) m -> p a m", p=128))
        w_sb[nm] = wt
    wo_sb = const.tile([128, 1024], F16)
    nc.sync.dma_start(out=wo_sb, in_=woT)
    rA = const.tile([128, 4096], F16)
    nc.sync.dma_start(out=rA, in_=ropeA)
    rB = const.tile([128, 4096], F16)
    nc.sync.dma_start(out=rB, in_=ropeB)
    ident = const.tile([128, 128], F16)
    make_identity(nc, ident)

    q_rot = const.tile([128, 4096], F16)
    k_rot = const.tile([128, 4096], F16)
    # per 128-token tile, per head: [V(0:64) | ones(64) | pad] fp16
    v_sb = [
        [const.tile([128, 72], F16, name=f"vsb{i}h{h}") for h in range(2)]
        for i in range(NTOK // 128)
    ]
    for vpair in v_sb:
        for vt in vpair:
            nc.vector.memset(vt, 1.0)

    at = tc.alloc_tile_pool(name="attn", bufs=1)
    pp = tc.alloc_tile_pool(name="phasep", bufs=1)

    # ---------------- phase P (emitted interleaved with attention) ----------
    xs = [pp.tile([128, 4096], F16, name=f"xs{k}") for k in range(8)]
    for t4 in range(4):
        for k in range(8):
            cs = slice(t4 * 1024, (t4 + 1) * 1024)
            nc.sync.dma_start(out=xs[k][:, cs], in_=xT[k * 128 : (k + 1) * 128, cs])
    vt_raw = pp.tile([128, 4096], F16)

    def proj_chunk(wt, dst, t4):
        cs = slice(t4 * 1024, (t4 + 1) * 1024)
        ps = psum.tile([128, 1024], F32, tag="mm", bufs=2, name="ps_pr")
        for k in range(8):
            for h2 in range(2):
                nc.tensor.matmul(
                    ps[:, h2 * 512 : (h2 + 1) * 512],
                    lhsT=wt[:, k, :],
                    rhs=xs[k][:, t4 * 1024 + h2 * 512 : t4 * 1024 + (h2 + 1) * 512],
                    start=(k == 0),
                    stop=(k == 7),
                )
        nc.vector.tensor_copy(dst[:, cs], ps)

    def rope_chunk(raw, t4):
        # out = raw*A + swap(raw)*B, swap = +-32 partitions within a head
        cs = slice(t4 * 1024, (t4 + 1) * 1024)
        sw = pp.tile([128, 1024], F16, tag="sw", bufs=2, name="ropesw")
        for dst_p, src_p in ((0, 32), (32, 0), (64, 96), (96, 64)):
            nc.sync.dma_start(
                out=sw[dst_p : dst_p + 32, :], in_=raw[src_p : src_p + 32, cs]
            )
        t1 = pp.tile([128, 1024], F16, tag="t1", bufs=2, name="ropet1")
        nc.vector.tensor_mul(t1, raw[:, cs], rA[:, cs])
        nc.vector.tensor_mul(sw, sw, rB[:, cs])
        nc.vector.tensor_add(raw[:, cs], t1, sw)

    def v_chunk_transpose(t4):
        # V transpose on the PE: vt_raw [d', tok] -> v_sb [tok128, d64]
        for i in range(8 * t4, 8 * (t4 + 1)):
            ts = slice(i * 128, (i + 1) * 128)
            pst = psum.tile([128, 1024], F32, tag="mm", bufs=2, name="ps_tr")
            tr = pst[:, 0:64].bitcast(F16)  # [128, 128] f16 view
            nc.tensor.transpose(tr, vt_raw[:, ts], ident)
            nc.vector.tensor_copy(v_sb[i][0][:, 0:64], tr[:, 0:64])
            nc.vector.tensor_copy(v_sb[i][1][:, 0:64], tr[:, 64:128])

    def phase_p(t4):
        proj_chunk(w_sb["wq"], q_rot, t4)
        rope_chunk(q_rot, t4)
        proj_chunk(w_sb["wk"], k_rot, t4)
        rope_chunk(k_rot, t4)
        proj_chunk(w_sb["wv"], vt_raw, t4)
        v_chunk_transpose(t4)

    # ---------------- attention ----------------
    pending_oproj = [None]

    def flush_oproj():
        if pending_oproj[0] is not None:
            pending_oproj[0]()
            pending_oproj[0] = None

    def chunk(b, qh):
        qoff = b * T + qh * QCH
        crow = 2 * b + qh  # chunk index 0..3

        exp_tiles = {}

        def s_exp(kt):
            # the two heads' score matmuls are row-tiled (PE rows 0:64 /
            # 64:128) and run concurrently when emitted adjacently.
            koff = b * T + kt * 128
            pss = [
                psum.tile([128, 1024], F32, tag="mm", bufs=2, name=f"ps_s{hi}")
                for hi in range(2)
            ]
            for h2 in range(2):
                for hi in range(2):
                    hs = slice(64 * hi, 64 * hi + 64)
                    nc.tensor.matmul(
                        pss[hi][:, h2 * 512 : (h2 + 1) * 512],
                        lhsT=k_rot[hs, koff : koff + 128],
                        rhs=q_rot[hs, qoff + h2 * 512 : qoff + (h2 + 1) * 512],
                        start=True,
                        stop=True,
                    )
            for hi in range(2):
                e = at.tile([128, 1024], F16, tag="exp", bufs=8, name="exps")
                nc.scalar.activation(e, pss[hi], Exp, scale=0.125)
                exp_tiles[(hi, kt)] = e

        ps_o = []

        def av(kt):
            vt = v_sb[b * KT_N + kt]
            for hi in range(2):
                e = exp_tiles.pop((hi, kt))
                for h2 in range(2):
                    h2s = slice(h2 * 512, (h2 + 1) * 512)
                    nc.tensor.matmul(
                        ps_o[hi][:, h2s],
                        lhsT=vt[hi][:, 0:65],
                        rhs=e[:, h2s],
                        start=(kt == 0),
                        stop=(kt == KT_N - 1),
                        skip_group_check=True,
                    )

        # pipelined emission with one-step lag so the PE never waits on exp;
        # ps_o is allocated after the previous chunk's oproj so the "o" psum
        # slots recycle without a chunk-long wait
        s_exp(0)
        s_exp(1)
        flush_oproj()
        ps_o.extend(
            psum.tile([65, 1024], F32, tag="o", bufs=2, name=f"ps_o{hi}")
            for hi in range(2)
        )
        av(0)
        for kt in range(2, KT_N):
            s_exp(kt)
            av(kt - 1)
        av(KT_N - 1)

        # evict unnormalized O^T per head + denominators
        ocat = at.tile([128, 1024], F16, tag="ocat", bufs=2, name="ocat")
        nc.vector.tensor_copy(ocat[0:64, :], ps_o[0][0:64, :])
        oBt = at.tile([64, 1024], F16, tag="oBt", bufs=2, name="oBt")
        nc.vector.tensor_copy(oBt, ps_o[1][0:64, :])
        nc.sync.dma_start(out=ocat[64:128, :], in_=oBt)
        for hi in range(2):
            dent = at.tile([1, 1024], F32, tag="dent", bufs=2, name="dent")
            nc.vector.tensor_copy(dent, ps_o[hi][64:65, :])
            nc.sync.dma_start(
                out=dens[2 * crow + hi : 2 * crow + hi + 1, :], in_=dent
            )

        def oproj():
            for nt in range(8):
                nts = slice(nt * 128, (nt + 1) * 128)
                ot = [
                    at.tile([128, 1024], F32, tag=f"ot{hi}", bufs=2, name=f"ot{hi}")
                    for hi in range(2)
                ]
                ps_u = [
                    psum.tile([128, 1024], F32, tag="o", bufs=2, name=f"ps_u{hi}")
                    for hi in range(2)
                ]
                for h2 in range(2):
                    h2s = slice(h2 * 512, (h2 + 1) * 512)
                    for hi in range(2):
                        hs = slice(64 * hi, 64 * hi + 64)
                        nc.tensor.matmul(
                            ps_u[hi][:, h2s],
                            lhsT=wo_sb[hs, nts],
                            rhs=ocat[hs, h2s],
                            start=True,
                            stop=True,
                        )
                for hi, outT in ((0, outTA), (1, outTB)):
                    nc.vector.tensor_copy(ot[hi], ps_u[hi])
                    nc.sync.dma_start(
                        out=outT[nts, qoff : qoff + QCH], in_=ot[hi]
                    )

        pending_oproj[0] = oproj

    # ---------------- schedule ----------------
    phase_p(0)
    phase_p(1)
    chunk(0, 0)
    phase_p(2)
    chunk(0, 1)
    phase_p(3)
    chunk(1, 0)
    chunk(1, 1)
    flush_oproj()

    pp.release()
    at.release()
    const.release()
    psum.release()


_NC_CACHE = {}


def _build_program():
    if 0 in _NC_CACHE:
        return _NC_CACHE[0]
    nc = bacc.Bacc("TRN2", num_devices=N_CORES, debug=False)
    xT = nc.dram_tensor("xT", [D, NTOK], F16, kind="ExternalInput").ap()
    wqT = nc.dram_tensor("wqT", [D, 128], F16, kind="ExternalInput").ap()
    wkT = nc.dram_tensor("wkT", [D, 128], F16, kind="ExternalInput").ap()
    wvT = nc.dram_tensor("wvT", [D, 128], F16, kind="ExternalInput").ap()
    woT = nc.dram_tensor("woT", [128, D], F16, kind="ExternalInput").ap()
    ropeA = nc.dram_tensor("ropeA", [128, NTOK], F16, kind="ExternalInput").ap()
    ropeB = nc.dram_tensor("ropeB", [128, NTOK], F16, kind="ExternalInput").ap()
    outTA = nc.dram_tensor("outTA", [D, NTOK], F32, kind="ExternalOutput").ap()
    outTB = nc.dram_tensor("outTB", [D, NTOK], F32, kind="ExternalOutput").ap()
    dens = nc.dram_tensor("dens", [8, QCH], F32, kind="ExternalOutput").ap()
    with tile.TileContext(nc) as tc:
        _build_body(tc, xT, wqT, wkT, wvT, woT, ropeA, ropeB, outTA, outTB, dens)
    nc.compile()
    _NC_CACHE[0] = nc
    return nc


def _rope_tables():
    half = DK // 2  # 32
    inv_freq = 1.0 / (
        10000.0 ** (np.arange(0, DK, 2, dtype=np.float32) / np.float32(DK))
    )
    t = np.arange(T, dtype=np.float32)
    freqs = np.outer(t, inv_freq)  # [T, 32]
    cos = np.cos(freqs)
    sin = np.sin(freqs)
    A = np.empty((128, NTOK), np.float32)
    Bt = np.empty((128, NTOK), np.float32)
    for p in range(128):
        i = p % DK
        if i < half:
            a, bb = cos[:, i], -sin[:, i]
        else:
            a, bb = cos[:, i - half], sin[:, i - half]
        for bi in range(B):
            A[p, bi * T : (bi + 1) * T] = a
            Bt[p, bi * T : (bi + 1) * T] = bb
    return A.astype(np.float16), Bt.astype(np.float16)


def _prep_inputs(x, wq, wk, wv, wo):
    xT = np.ascontiguousarray(x.reshape(NTOK, D).T).astype(np.float16)
    ropeA, ropeB = _rope_tables()
    in_maps = []
    for c in range(N_CORES):
        rows = slice(128 * c, 128 * (c + 1))
        in_maps.append(
            {
                "xT": xT,
                "wqT": np.ascontiguousarray(wq[rows, :].T).astype(np.float16),
                "wkT": np.ascontiguousarray(wk[rows, :].T).astype(np.float16),
                "wvT": np.ascontiguousarray(wv[rows, :].T).astype(np.float16),
                "woT": np.ascontiguousarray(wo[:, rows].T).astype(np.float16),
                "ropeA": ropeA,
                "ropeB": ropeB,
            }
        )
    return in_maps


def run(x, wq, wk, wv, wo, trace=False):
    """Returns (output (B,T,D) fp32, BassKernelResults)."""
    from concourse import bass_utils

    nc = _build_program()
    in_maps = _prep_inputs(
        np.asarray(x, np.float32),
        np.asarray(wq, np.float32),
        np.asarray(wk, np.float32),
        np.asarray(wv, np.float32),
        np.asarray(wo, np.float32),
    )
    res = bass_utils.run_bass_kernel_spmd(
        nc, in_maps, core_ids=list(range(N_CORES)), trace=trace
    )
    acc = np.zeros((D, NTOK), np.float32)
    for c in range(N_CORES):
        r = res.results[c]
        dens_ = np.asarray(r["dens"], np.float32)  # [8, 1024]
        rec = np.empty((2, NTOK), np.float32)
        for b in range(B):
            for qh in range(2):
                crow = 2 * b + qh
                qoff = b * T + qh * QCH
                for hi in range(2):
                    rec[hi, qoff : qoff + QCH] = 1.0 / dens_[2 * crow + hi]
        acc += np.asarray(r["outTA"], np.float32) * rec[0][None, :]
        acc += np.asarray(r["outTB"], np.float32) * rec[1][None, :]
    out = acc.T.reshape(B, T, D)
    return out, res


def kernel(x, wq, wk, wv, wo):
    out, _ = run(x, wq, wk, wv, wo)
    return out
